# revision 4
# baseline (speedup 1.0000x reference)
"""4-layer GCN on 8 Trainium2 NeuronCores.

Strategy (destination/node sharding):
  - Nodes row-sharded across 8 cores (6250 rows each). Edges owned by their
    destination core. Weights replicated.
  - Per layer L in {0,1,2}:  P = h @ W_L (local GEMM, bf16),
    S = dis * P  (dis = deg^-1/2, row scale, bf16)  -> AllGather(S) ->
    agg_d = sum_{e: dst=d} S[src_e]   (gather + one-hot matmul accumulate)
    h' = relu(dis_d * agg_d + b_L)
    Layer 3 commutes the GEMM past the aggregation (OUT_DIM=2 gathers would be
    tiny/inefficient): S = dis*h3, agg, out = (dis_d*agg) @ W3 + b3.
  - The edge aggregation runs as: dma_gather of 128-row chunks of S (bf16,
    512B rows, full DMA rate), then PE matmul  psum += M_chunk^T @ msgs_chunk
    where M[p, f] = (dstloc[p] == f) is built on the DVE from an iota +
    int16 compare (edges pre-sorted by destination block on the host).
  - AllGather is split into 4 block-aligned row slices per layer so the edge
    pass pipelines against the collective, and so each gather region stays
    under the int16 index limit of dma_gather.
"""

import math
import numpy as np
import ml_dtypes

BF16 = ml_dtypes.bfloat16

# ---------------------------------------------------------------- config ----


def make_cfg(n, e, d, dout, n_cores, slice_blocks):
    blk = 128
    nloc = n // n_cores
    nb = math.ceil(nloc / blk)
    assert slice_blocks[0] == 0 and slice_blocks[-1] == nb
    row_starts = [min(b * blk, nloc) for b in slice_blocks]
    slice_lens = [row_starts[i + 1] - row_starts[i] for i in range(len(row_starts) - 1)]
    return dict(
        N=n, E=e, D=d, DOUT=dout, C=n_cores, BLK=blk, NLOC=nloc, NB=nb,
        NPAD=nb * blk,
        SLICE_BLOCKS=slice_blocks,          # block index bounds per slice
        ROW_STARTS=row_starts[:-1],         # local-row start per slice
        SLICE_LENS=slice_lens,              # local rows per slice
        NS=len(slice_lens),
        GATHER_CHUNKS=8,                   # chunks (of 128 edges) per dma_gather
        MB=8,                               # chunks per M-build batch
    )


CFG = make_cfg(50000, 800000, 256, 2, 8, [0, 13, 25, 37, 49])

# ---------------------------------------------------------- host preprocess --


def preprocess(cfg, edge_index):
    """Sort/pad edges per core; build gather-index + dstloc streams.

    Returns (shared_meta, per_core_arrays).
    """
    N, C, NLOC, BLK, NB, NS = (cfg[k] for k in ("N", "C", "NLOC", "BLK", "NB", "NS"))
    row_starts = np.array(cfg["ROW_STARTS"], np.int64)
    slice_lens = np.array(cfg["SLICE_LENS"], np.int64)

    src = np.concatenate([np.asarray(edge_index[0], np.int64), np.arange(N)])
    dst = np.concatenate([np.asarray(edge_index[1], np.int64), np.arange(N)])
    deg = np.bincount(dst, minlength=N).astype(np.float32)
    dis = deg ** -0.5

    core = dst // NLOC
    per_core_raw = []
    counts = np.zeros((C, NS, NB), np.int64)
    for c in range(C):
        m = core == c
        s_e, d_e = src[m], dst[m] - c * NLOC
        b_e = d_e // BLK
        loc_e = d_e % BLK
        o_e = s_e // NLOC
        r_e = s_e % NLOC
        sl_e = np.searchsorted(row_starts, r_e, side="right") - 1
        gidx = o_e * slice_lens[sl_e] + (r_e - row_starts[sl_e])
        key = sl_e * NB + b_e
        order = np.argsort(key, kind="stable")
        per_core_raw.append((key[order], gidx[order], loc_e[order]))
        cnt = np.bincount(key, minlength=NS * NB)
        counts[c] = cnt.reshape(NS, NB)

    # uniform chunk counts across cores (SPMD: one program)
    n_chunks = np.maximum(1, -(-counts.max(axis=0) // BLK))  # [NS, NB]
    chunks_per_slice = n_chunks.sum(axis=1)                  # [NS]
    tch = int(chunks_per_slice.sum())
    total_idx = tch * BLK

    # chunk-group offsets (in chunks) per (s, b)
    group_off = np.zeros((NS, NB), np.int64)
    acc = 0
    for s in range(NS):
        for b in range(NB):
            group_off[s, b] = acc
            acc += n_chunks[s, b]

    # gather instruction split: per slice, pieces of <= GATHER_CHUNKS chunks
    gather_insts = []  # (slice, chunk_off, n_chunk)
    for s in range(NS):
        start = int(group_off[s, 0])
        rem = int(chunks_per_slice[s])
        off = start
        while rem > 0:
            n = min(rem, cfg["GATHER_CHUNKS"])
            gather_insts.append((s, off, n))
            off += n
            rem -= n

    per_core = []
    for c in range(C):
        key_s, gidx_s, loc_s = per_core_raw[c]
        idx_stream = np.zeros(total_idx, np.int16)
        dloc_stream = np.full(total_idx, -1, np.int16)
        # place each (s,b) group at its padded offset
        grp_sizes = np.bincount(key_s, minlength=NS * NB)
        pos = 0
        for g in range(NS * NB):
            sz = int(grp_sizes[g])
            s_, b_ = g // NB, g % NB
            o0 = int(group_off[s_, b_]) * BLK
            idx_stream[o0:o0 + sz] = gidx_s[pos:pos + sz]
            dloc_stream[o0:o0 + sz] = loc_s[pos:pos + sz]
            pos += sz
        # wrap: position i -> [i % 16, i // 16], replicate to 128 partitions
        idx_w = np.tile(idx_stream.reshape(-1, 16).T, (8, 1))  # [128, total/16]
        # dstloc: chunk j, partition p  -> edge j*128+p
        dloc_w = dloc_stream.reshape(-1, BLK).T.copy()         # [128, tch]
        per_core.append(dict(idx=idx_w, dstloc=dloc_w))

    meta = dict(n_chunks=n_chunks, group_off=group_off, gather_insts=gather_insts,
                tch=tch, total_idx=total_idx, dis=dis)
    return meta, per_core


def make_in_maps(cfg, meta, per_core, x, W, B):
    C, NLOC, NB, BLK, NPAD, D, DOUT = (cfg[k] for k in
                                       ("C", "NLOC", "NB", "BLK", "NPAD", "D", "DOUT"))
    dis = meta["dis"]
    in_maps = []
    for c in range(C):
        xc = np.asarray(x[c * NLOC:(c + 1) * NLOC], np.float32)
        xT = np.zeros((D, NPAD), BF16)
        xT[:, :NLOC] = xc.T.astype(BF16)
        dis_c = dis[c * NLOC:(c + 1) * NLOC]
        dis_w = np.ones((BLK, NB), np.float32)
        dw = np.ones(NPAD, np.float32)
        dw[:NLOC] = dis_c
        dis_w[:, :] = dw.reshape(NB, BLK).T
        invdis = np.zeros((1, NPAD), np.float32)
        invdis[0, :NLOC] = 1.0 / dis_c
        m = dict(
            xT=xT,
            idx=per_core[c]["idx"],
            dstloc=per_core[c]["dstloc"],
            disw=dis_w,
            invdis=invdis,
            ones=np.ones((1, BLK), np.float32),
            w0=np.asarray(W[0], np.float32).astype(BF16),
            w1=np.asarray(W[1], np.float32).astype(BF16),
            w2=np.asarray(W[2], np.float32).astype(BF16),
            w3=np.asarray(W[3], np.float32).astype(BF16),
            b0=np.asarray(B[0], np.float32).reshape(1, D),
            b1=np.asarray(B[1], np.float32).reshape(1, D),
            b2=np.asarray(B[2], np.float32).reshape(1, D),
            b3=np.asarray(B[3], np.float32).reshape(1, DOUT),
        )
        in_maps.append(m)
    return in_maps


# ---------------------------------------------------------------- builder ----


def build(cfg, meta):
    import sys
    if "/opt/trn_rl_repo" not in sys.path:
        sys.path.insert(0, "/opt/trn_rl_repo")
    import concourse.bass as bass
    from concourse import bacc, tile, mybir

    C, D, DOUT, BLK, NB, NS, NLOC, NPAD = (cfg[k] for k in
        ("C", "D", "DOUT", "BLK", "NB", "NS", "NLOC", "NPAD"))
    MB, GCH = cfg["MB"], cfg["GATHER_CHUNKS"]
    n_chunks, group_off = meta["n_chunks"], meta["group_off"]
    gather_insts, tch = meta["gather_insts"], meta["tch"]
    slice_lens, slice_blocks = cfg["SLICE_LENS"], cfg["SLICE_BLOCKS"]
    NFI = D // BLK  # f_in halves (2)
    f32, bf16, i16 = mybir.dt.float32, mybir.dt.bfloat16, mybir.dt.int16
    FT = mybir.ActivationFunctionType
    OP = mybir.AluOpType
    RG = [list(range(C))]

    nc = bacc.Bacc("TRN2", target_bir_lowering=False, debug=False, num_devices=C)

    xT = nc.dram_tensor("xT", [D, NPAD], bf16, kind="ExternalInput")
    idx_d = nc.dram_tensor("idx", [128, meta["total_idx"] // 16], i16, kind="ExternalInput")
    dloc_d = nc.dram_tensor("dstloc", [128, tch], i16, kind="ExternalInput")
    disw_d = nc.dram_tensor("disw", [BLK, NB], f32, kind="ExternalInput")
    invdis_d = nc.dram_tensor("invdis", [1, NPAD], f32, kind="ExternalInput")
    ones_d = nc.dram_tensor("ones", [1, BLK], f32, kind="ExternalInput")
    w_d = [nc.dram_tensor(f"w{l}", [D, D if l < 3 else DOUT], bf16, kind="ExternalInput")
           for l in range(4)]
    b_d = [nc.dram_tensor(f"b{l}", [1, D if l < 3 else DOUT], f32, kind="ExternalInput")
           for l in range(4)]
    outT = nc.dram_tensor("outT", [DOUT, NLOC], f32, kind="ExternalOutput")

    last_rows = NLOC - (NB - 1) * BLK  # rows in final block

    with tile.TileContext(nc) as tc:
        with tc.tile_pool(name="const", bufs=1) as pc, \
             tc.tile_pool(name="hpool", bufs=1) as ph, \
             tc.tile_pool(name="aggpool", bufs=1) as pa, \
             tc.tile_pool(name="work", bufs=3) as pw, \
             tc.tile_pool(name="gpool", bufs=2) as pg, \
             tc.tile_pool(name="mpool", bufs=3) as pm, \
             tc.tile_pool(name="psA", bufs=3, space="PSUM") as ppa, \
             tc.tile_pool(name="psT", bufs=2, space="PSUM") as ppt, \
             tc.tile_pool(name="psG", bufs=2, space="PSUM") as ppg, \
             tc.tile_pool(name="dram", bufs=2, space="DRAM") as pd:

            # ---- constants into SBUF
            idx_t = pc.tile([128, meta["total_idx"] // 16], i16, tag="idx")
            nc.sync.dma_start(out=idx_t[:], in_=idx_d[:])
            dloc_t = pc.tile([128, tch], i16, tag="dloc")
            nc.sync.dma_start(out=dloc_t[:], in_=dloc_d[:])
            dis_t = pc.tile([BLK, NB], f32, tag="disw")
            nc.sync.dma_start(out=dis_t[:], in_=disw_d[:])
            invdis_t = pc.tile([1, NPAD], f32, tag="invdis")
            nc.sync.dma_start(out=invdis_t[:], in_=invdis_d[:])
            ones_t = pc.tile([1, BLK], f32, tag="ones")
            nc.sync.dma_start(out=ones_t[:], in_=ones_d[:])
            w_t = []
            for l in range(3):
                tiles = []
                for fi in range(NFI):
                    t = pc.tile([BLK, D], bf16, tag=f"w{l}_{fi}")
                    nc.sync.dma_start(out=t[:], in_=w_d[l][fi * BLK:(fi + 1) * BLK, :])
                    tiles.append(t)
                w_t.append(tiles)
            w3_t = []
            for fi in range(NFI):
                t = pc.tile([BLK, DOUT], bf16, tag=f"w3_{fi}")
                nc.sync.dma_start(out=t[:], in_=w_d[3][fi * BLK:(fi + 1) * BLK, :])
                w3_t.append(t)
            b_t = []
            for l in range(4):
                t = pc.tile([1, D if l < 3 else DOUT], f32, tag=f"b{l}")
                nc.sync.dma_start(out=t[:], in_=b_d[l][:])
                b_t.append(t)

            # iota [128, MB, 128] (value = inner index) and identity matrix
            iota_t = pc.tile([128, MB, BLK], i16, tag="iota")
            nc.gpsimd.iota(iota_t[:], pattern=[[0, MB], [1, BLK]], base=0,
                           channel_multiplier=0)
            pm_t = pc.tile([128, BLK], i16, tag="pmf")
            nc.gpsimd.iota(pm_t[:], pattern=[[-1, BLK]], base=0, channel_multiplier=1)
            id_bf = pc.tile([128, BLK], bf16, tag="idbf")
            nc.vector.tensor_scalar(id_bf[:], pm_t[:], 0, None, OP.is_equal)

            outsb = pc.tile([DOUT, NLOC], f32, tag="outsb")

            h_t = [None] * NB      # current-layer activations per block (bf16)
            agg_t = [None] * NB    # f32 aggregation accumulators

            def rows_of(b):
                return last_rows if b == NB - 1 else BLK

            def slice_of_block(b):
                for s in range(NS):
                    if slice_blocks[s] <= b < slice_blocks[s + 1]:
                        return s
                raise AssertionError

            # regions / ag inputs per layer generation (pool bufs=2 rotates)
            def new_comm_tiles():
                agin = [pd.tile([slice_lens[s], D], bf16, tag=f"agin{s}", name=f"agin{s}")
                        for s in range(NS)]
                region = [pd.tile([C * slice_lens[s], D], bf16, tag=f"region{s}",
                                  addr_space="Shared", name=f"region{s}") for s in range(NS)]
                return agin, region

            def emit_gemm_scale(l, b, agin):
                """S_l[b] = dis_b * (h @ W_l)  (bf16) -> DMA into agin slice.
                l == 3 skips the GEMM (S_3 = dis*h3)."""
                s = slice_of_block(b)
                r0 = (b - slice_blocks[s]) * BLK
                rows = rows_of(b)
                sblk = pw.tile([BLK, D], bf16, tag="sblk")
                if l == 3:
                    nc.scalar.activation(sblk[:], h_t[b][:], FT.Copy,
                                         scale=dis_t[:, b:b + 1])
                else:
                    # lhsT tiles [f_in_half, n]
                    if l == 0:
                        hT = []
                        for fi in range(NFI):
                            t = pw.tile([BLK, BLK], bf16, tag=f"hT{fi}")
                            nc.sync.dma_start(
                                out=t[:],
                                in_=xT[fi * BLK:(fi + 1) * BLK, b * BLK:(b + 1) * BLK])
                            hT.append(t)
                    else:
                        hT = []
                        for fi in range(NFI):
                            tp = ppt.tile([BLK, BLK], bf16, tag="tp")
                            nc.tensor.transpose(tp[:], h_t[b][:, fi * BLK:(fi + 1) * BLK],
                                                id_bf[:])
                            t = pw.tile([BLK, BLK], bf16, tag=f"hT{fi}")
                            nc.scalar.copy(t[:], tp[:])
                            hT.append(t)
                    gp = ppg.tile([BLK, D], f32, tag="gp")
                    for fi in range(NFI):
                        nc.tensor.matmul(gp[:], hT[fi][:], w_t[l][fi][:],
                                         start=(fi == 0), stop=(fi == NFI - 1))
                    nc.scalar.activation(sblk[:], gp[:], FT.Copy,
                                         scale=dis_t[:, b:b + 1])
                nc.sync.dma_start(out=agin[s][r0:r0 + rows, :], in_=sblk[:rows, :])

            def emit_ag(s, agin, region):
                nc.gpsimd.collective_compute(
                    "AllGather", OP.bypass, replica_groups=RG,
                    ins=[agin[s][:]], outs=[region[s][:]])

            # ---------------- layer 0 phase A
            agin_cur, region_cur = new_comm_tiles()
            for b in range(NB):
                emit_gemm_scale(0, b, agin_cur)
                if b + 1 in slice_blocks:
                    emit_ag(slice_of_block(b), agin_cur, region_cur)

            # ---------------- layers: edge pass (+ fused next-layer prep)
            for l in range(4):
                agin_next, region_next = (new_comm_tiles() if l < 3 else (None, None))
                # gather + M-build + matmul streams
                g_tiles = {}   # chunk_off -> (tile, chunk_off, n)
                for (s, coff, n) in gather_insts:
                    gt = pg.tile([128, n, D], bf16, tag="g")
                    nc.gpsimd.dma_gather(
                        out_ap=gt[:],
                        in_ap=region_cur[s][:],
                        idxs_ap=idx_t[:, coff * 8:(coff + n) * 8],
                        num_idxs=n * BLK,
                        num_idxs_reg=n * BLK,
                        elem_size=D,
                    )
                    g_tiles[coff] = (gt, coff, n)
                m_tiles = {}   # mb batch index -> tile
                n_mb = -(-tch // MB)
                for k in range(n_mb):
                    nchk = min(MB, tch - k * MB)
                    mt = pm.tile([128, nchk, BLK], bf16, tag="m")
                    nc.vector.tensor_tensor(
                        mt[:],
                        iota_t[:, :nchk, :],
                        dloc_t[:, k * MB:k * MB + nchk]
                            .broadcast_to([128, nchk, BLK]),
                        OP.is_equal)
                    m_tiles[k] = mt

                def g_slice(j):
                    for (gt, coff, n) in g_tiles.values():
                        if coff <= j < coff + n:
                            return gt[:, j - coff, :]
                    raise AssertionError

                def m_slice(j):
                    return m_tiles[j // MB][:, j % MB, :]

                for s in range(NS):
                    for b in range(NB):
                        c0 = int(group_off[s, b])
                        ncks = int(n_chunks[s, b])
                        ap_ = ppa.tile([BLK, D], f32, tag="aggp")
                        for j in range(c0, c0 + ncks):
                            is_last = (j == c0 + ncks - 1)
                            add_bias = is_last and s == NS - 1 and l < 3
                            nc.tensor.matmul(ap_[:], m_slice(j), g_slice(j),
                                             start=(j == c0),
                                             stop=(is_last and not add_bias))
                            if add_bias:
                                nc.tensor.matmul(
                                    ap_[:],
                                    invdis_t[0:1, b * BLK:(b + 1) * BLK],
                                    b_t[l][:], start=False, stop=True)
                        if s == 0:
                            agg_t[b] = pa.tile([BLK, D], f32, tag=f"agg{b}", name=f"agg{b}")
                            nc.vector.tensor_copy(agg_t[b][:], ap_[:])
                        else:
                            nc.vector.tensor_add(agg_t[b][:], agg_t[b][:], ap_[:])

                        if s == NS - 1:
                            # epilogue for block b
                            if l < 3:
                                h_t[b] = ph.tile([BLK, D], bf16, tag=f"h{b}", name=f"h{b}")
                                nc.scalar.activation(h_t[b][:], agg_t[b][:], FT.Relu,
                                                     scale=dis_t[:, b:b + 1])
                                emit_gemm_scale(l + 1, b, agin_next)
                                if b + 1 in slice_blocks:
                                    emit_ag(slice_of_block(b), agin_next, region_next)
                            else:
                                tblk = pw.tile([BLK, D], bf16, tag="sblk")
                                nc.scalar.activation(tblk[:], agg_t[b][:], FT.Copy,
                                                     scale=dis_t[:, b:b + 1])
                                tT = []
                                for fi in range(NFI):
                                    tp = ppt.tile([BLK, BLK], bf16, tag="tp")
                                    nc.tensor.transpose(
                                        tp[:], tblk[:, fi * BLK:(fi + 1) * BLK], id_bf[:])
                                    t = pw.tile([BLK, BLK], bf16, tag=f"hT{fi}")
                                    nc.scalar.copy(t[:], tp[:])
                                    tT.append(t)
                                op_ = ppa.tile([DOUT, BLK], f32, tag="aggp")
                                for fi in range(NFI):
                                    nc.tensor.matmul(op_[:], w3_t[fi][:], tT[fi][:],
                                                     start=(fi == 0), stop=False)
                                nc.tensor.matmul(op_[:], b_t[3][:], ones_t[:],
                                                 start=False, stop=True)
                                rows = rows_of(b)
                                nc.scalar.copy(
                                    outsb[:, b * BLK:b * BLK + rows], op_[:, :rows])
                if l < 3:
                    agin_cur, region_cur = agin_next, region_next

            nc.sync.dma_start(out=outT[:], in_=outsb[:])

    nc.compile()
    return nc


# ----------------------------------------------------------------- driver ----


def _gather_full_inputs(cfg, inputs):
    x = np.asarray(inputs["x"], np.float32)
    W = [np.asarray(inputs[f"W{l}"], np.float32) for l in range(4)]
    B = [np.asarray(inputs[f"b{l}"], np.float32) for l in range(4)]
    return x, W, B


def run(cfg, inputs, runner):
    """runner(nc, in_maps) -> list of {name: np.ndarray} per core."""
    meta, per_core = preprocess(cfg, np.asarray(inputs["edge_index"]))
    x, W, B = _gather_full_inputs(cfg, inputs)
    in_maps = make_in_maps(cfg, meta, per_core, x, W, B)
    nc = build(cfg, meta)
    results = runner(nc, in_maps)
    outs = [np.asarray(r["outT"], np.float32).T for r in results]  # [NLOC, DOUT]
    return np.concatenate(outs, axis=0)


def _hw_runner(nc, in_maps):
    import sys
    if "/opt/trn_rl_repo" not in sys.path:
        sys.path.insert(0, "/opt/trn_rl_repo")
    from concourse import bass_utils
    res = bass_utils.run_bass_kernel_spmd(nc, in_maps, core_ids=list(range(len(in_maps))))
    return res.results


def kernel(**inputs):
    return run(CFG, inputs, _hw_runner)


# revision 6
# speedup vs baseline: 1.0360x; 1.0360x over previous
"""4-layer GCN on 8 Trainium2 NeuronCores.

Strategy (destination/node sharding):
  - Nodes row-sharded across 8 cores (6250 rows each). Edges owned by their
    destination core. Weights replicated.
  - Per layer L in {0,1,2}:  P = h @ W_L (local GEMM, bf16),
    S = dis * P  (dis = deg^-1/2, row scale, bf16)  -> AllGather(S) ->
    agg_d = sum_{e: dst=d} S[src_e]   (gather + one-hot matmul accumulate)
    h' = relu(dis_d * agg_d + b_L)
    Layer 3 commutes the GEMM past the aggregation (OUT_DIM=2 gathers would be
    tiny/inefficient): S = dis*h3, agg, out = (dis_d*agg) @ W3 + b3.
  - The edge aggregation runs as: dma_gather of 128-row chunks of S (bf16,
    512B rows, full DMA rate), then PE matmul  psum += M_chunk^T @ msgs_chunk
    where M[p, f] = (dstloc[p] == f) is built on the DVE from an iota +
    int16 compare (edges pre-sorted by destination block on the host).
  - AllGather is split into 4 block-aligned row slices per layer so the edge
    pass pipelines against the collective, and so each gather region stays
    under the int16 index limit of dma_gather.
"""

import math
import numpy as np
import ml_dtypes

BF16 = ml_dtypes.bfloat16

# ---------------------------------------------------------------- config ----


def make_cfg(n, e, d, dout, n_cores, slice_blocks):
    blk = 128
    nloc = n // n_cores
    nb = math.ceil(nloc / blk)
    assert slice_blocks[0] == 0 and slice_blocks[-1] == nb
    row_starts = [min(b * blk, nloc) for b in slice_blocks]
    slice_lens = [row_starts[i + 1] - row_starts[i] for i in range(len(row_starts) - 1)]
    return dict(
        N=n, E=e, D=d, DOUT=dout, C=n_cores, BLK=blk, NLOC=nloc, NB=nb,
        NPAD=nb * blk,
        SLICE_BLOCKS=slice_blocks,          # block index bounds per slice
        ROW_STARTS=row_starts[:-1],         # local-row start per slice
        SLICE_LENS=slice_lens,              # local rows per slice
        NS=len(slice_lens),
        GATHER_CHUNKS=8,                   # chunks (of 128 edges) per dma_gather
        MB=8,                               # chunks per M-build batch
    )


CFG = make_cfg(50000, 800000, 256, 2, 8, [0, 13, 25, 37, 49])

# ---------------------------------------------------------- host preprocess --


def preprocess(cfg, edge_index):
    """Sort/pad edges per core; build gather-index + dstloc streams.

    Returns (shared_meta, per_core_arrays).
    """
    N, C, NLOC, BLK, NB, NS = (cfg[k] for k in ("N", "C", "NLOC", "BLK", "NB", "NS"))
    row_starts = np.array(cfg["ROW_STARTS"], np.int64)
    slice_lens = np.array(cfg["SLICE_LENS"], np.int64)

    src = np.concatenate([np.asarray(edge_index[0], np.int64), np.arange(N)])
    dst = np.concatenate([np.asarray(edge_index[1], np.int64), np.arange(N)])
    deg = np.bincount(dst, minlength=N).astype(np.float32)
    dis = deg ** -0.5

    core = dst // NLOC
    per_core_raw = []
    counts = np.zeros((C, NS, NB), np.int64)
    for c in range(C):
        m = core == c
        s_e, d_e = src[m], dst[m] - c * NLOC
        b_e = d_e // BLK
        loc_e = d_e % BLK
        o_e = s_e // NLOC
        r_e = s_e % NLOC
        sl_e = np.searchsorted(row_starts, r_e, side="right") - 1
        gidx = o_e * slice_lens[sl_e] + (r_e - row_starts[sl_e])
        key = sl_e * NB + b_e
        order = np.argsort(key, kind="stable")
        per_core_raw.append((key[order], gidx[order], loc_e[order]))
        cnt = np.bincount(key, minlength=NS * NB)
        counts[c] = cnt.reshape(NS, NB)

    # uniform chunk counts across cores (SPMD: one program)
    n_chunks = np.maximum(1, -(-counts.max(axis=0) // BLK))  # [NS, NB]
    chunks_per_slice = n_chunks.sum(axis=1)                  # [NS]
    tch = int(chunks_per_slice.sum())
    total_idx = tch * BLK

    # chunk-group offsets (in chunks) per (s, b)
    group_off = np.zeros((NS, NB), np.int64)
    acc = 0
    for s in range(NS):
        for b in range(NB):
            group_off[s, b] = acc
            acc += n_chunks[s, b]

    # gather instruction split: per slice, pieces of <= GATHER_CHUNKS chunks
    gather_insts = []  # (slice, chunk_off, n_chunk)
    for s in range(NS):
        start = int(group_off[s, 0])
        rem = int(chunks_per_slice[s])
        off = start
        while rem > 0:
            n = min(rem, cfg["GATHER_CHUNKS"])
            gather_insts.append((s, off, n))
            off += n
            rem -= n

    per_core = []
    for c in range(C):
        key_s, gidx_s, loc_s = per_core_raw[c]
        idx_stream = np.zeros(total_idx, np.int16)
        dloc_stream = np.full(total_idx, -1, np.int16)
        # place each (s,b) group at its padded offset
        grp_sizes = np.bincount(key_s, minlength=NS * NB)
        pos = 0
        for g in range(NS * NB):
            sz = int(grp_sizes[g])
            s_, b_ = g // NB, g % NB
            o0 = int(group_off[s_, b_]) * BLK
            idx_stream[o0:o0 + sz] = gidx_s[pos:pos + sz]
            dloc_stream[o0:o0 + sz] = loc_s[pos:pos + sz]
            pos += sz
        # wrap: position i -> [i % 16, i // 16], replicate to 128 partitions
        idx_w = np.tile(idx_stream.reshape(-1, 16).T, (8, 1))  # [128, total/16]
        # dstloc: chunk j, partition p  -> edge j*128+p
        dloc_w = dloc_stream.reshape(-1, BLK).T.copy()         # [128, tch]
        per_core.append(dict(idx=idx_w, dstloc=dloc_w))

    meta = dict(n_chunks=n_chunks, group_off=group_off, gather_insts=gather_insts,
                tch=tch, total_idx=total_idx, dis=dis)
    return meta, per_core


def make_in_maps(cfg, meta, per_core, x, W, B):
    C, NLOC, NB, BLK, NPAD, D, DOUT = (cfg[k] for k in
                                       ("C", "NLOC", "NB", "BLK", "NPAD", "D", "DOUT"))
    dis = meta["dis"]
    in_maps = []
    for c in range(C):
        xc = np.asarray(x[c * NLOC:(c + 1) * NLOC], np.float32)
        xT = np.zeros((D, NPAD), BF16)
        xT[:, :NLOC] = xc.T.astype(BF16)
        dis_c = dis[c * NLOC:(c + 1) * NLOC]
        dis_w = np.ones((BLK, NB), np.float32)
        dw = np.ones(NPAD, np.float32)
        dw[:NLOC] = dis_c
        dis_w[:, :] = dw.reshape(NB, BLK).T
        invdis = np.zeros((1, NPAD), np.float32)
        invdis[0, :NLOC] = 1.0 / dis_c
        m = dict(
            xT=xT,
            idx=per_core[c]["idx"],
            dstloc=per_core[c]["dstloc"],
            disw=dis_w,
            invdis=invdis,
            ones=np.ones((1, BLK), np.float32),
            w0=np.asarray(W[0], np.float32).astype(BF16),
            w1=np.asarray(W[1], np.float32).astype(BF16),
            w2=np.asarray(W[2], np.float32).astype(BF16),
            w3=np.asarray(W[3], np.float32).astype(BF16),
            b0=np.asarray(B[0], np.float32).reshape(1, D),
            b1=np.asarray(B[1], np.float32).reshape(1, D),
            b2=np.asarray(B[2], np.float32).reshape(1, D),
            b3=np.asarray(B[3], np.float32).reshape(1, DOUT),
        )
        in_maps.append(m)
    return in_maps


# ---------------------------------------------------------------- builder ----


def build(cfg, meta):
    import sys
    if "/opt/trn_rl_repo" not in sys.path:
        sys.path.insert(0, "/opt/trn_rl_repo")
    import concourse.bass as bass
    from concourse import bacc, tile, mybir

    C, D, DOUT, BLK, NB, NS, NLOC, NPAD = (cfg[k] for k in
        ("C", "D", "DOUT", "BLK", "NB", "NS", "NLOC", "NPAD"))
    MB, GCH = cfg["MB"], cfg["GATHER_CHUNKS"]
    n_chunks, group_off = meta["n_chunks"], meta["group_off"]
    gather_insts, tch = meta["gather_insts"], meta["tch"]
    slice_lens, slice_blocks = cfg["SLICE_LENS"], cfg["SLICE_BLOCKS"]
    NFI = D // BLK  # f_in halves (2)
    f32, bf16, i16 = mybir.dt.float32, mybir.dt.bfloat16, mybir.dt.int16
    FT = mybir.ActivationFunctionType
    OP = mybir.AluOpType
    RG = [list(range(C))]

    nc = bacc.Bacc("TRN2", target_bir_lowering=False, debug=False, num_devices=C)

    xT = nc.dram_tensor("xT", [D, NPAD], bf16, kind="ExternalInput")
    idx_d = nc.dram_tensor("idx", [128, meta["total_idx"] // 16], i16, kind="ExternalInput")
    dloc_d = nc.dram_tensor("dstloc", [128, tch], i16, kind="ExternalInput")
    disw_d = nc.dram_tensor("disw", [BLK, NB], f32, kind="ExternalInput")
    invdis_d = nc.dram_tensor("invdis", [1, NPAD], f32, kind="ExternalInput")
    ones_d = nc.dram_tensor("ones", [1, BLK], f32, kind="ExternalInput")
    w_d = [nc.dram_tensor(f"w{l}", [D, D if l < 3 else DOUT], bf16, kind="ExternalInput")
           for l in range(4)]
    b_d = [nc.dram_tensor(f"b{l}", [1, D if l < 3 else DOUT], f32, kind="ExternalInput")
           for l in range(4)]
    outT = nc.dram_tensor("outT", [DOUT, NLOC], f32, kind="ExternalOutput")

    last_rows = NLOC - (NB - 1) * BLK  # rows in final block

    with tile.TileContext(nc) as tc:
        with tc.tile_pool(name="const", bufs=1) as pc, \
             tc.tile_pool(name="hpool", bufs=1) as ph, \
             tc.tile_pool(name="aggpool", bufs=1) as pa, \
             tc.tile_pool(name="work", bufs=3) as pw, \
             tc.tile_pool(name="gpool", bufs=2) as pg, \
             tc.tile_pool(name="mpool", bufs=3) as pm, \
             tc.tile_pool(name="psA", bufs=3, space="PSUM") as ppa, \
             tc.tile_pool(name="psT", bufs=2, space="PSUM") as ppt, \
             tc.tile_pool(name="psG", bufs=2, space="PSUM") as ppg, \
             tc.tile_pool(name="dram", bufs=2, space="DRAM") as pd:

            # ---- constants into SBUF
            idx_t = pc.tile([128, meta["total_idx"] // 16], i16, tag="idx")
            nc.sync.dma_start(out=idx_t[:], in_=idx_d[:])
            dloc_t = pc.tile([128, tch], i16, tag="dloc")
            nc.sync.dma_start(out=dloc_t[:], in_=dloc_d[:])
            dis_t = pc.tile([BLK, NB], f32, tag="disw")
            nc.sync.dma_start(out=dis_t[:], in_=disw_d[:])
            invdis_t = pc.tile([1, NPAD], f32, tag="invdis")
            nc.sync.dma_start(out=invdis_t[:], in_=invdis_d[:])
            ones_t = pc.tile([1, BLK], f32, tag="ones")
            nc.sync.dma_start(out=ones_t[:], in_=ones_d[:])
            w_t = []
            for l in range(3):
                tiles = []
                for fi in range(NFI):
                    t = pc.tile([BLK, D], bf16, tag=f"w{l}_{fi}")
                    nc.sync.dma_start(out=t[:], in_=w_d[l][fi * BLK:(fi + 1) * BLK, :])
                    tiles.append(t)
                w_t.append(tiles)
            w3_t = []
            for fi in range(NFI):
                t = pc.tile([BLK, DOUT], bf16, tag=f"w3_{fi}")
                nc.sync.dma_start(out=t[:], in_=w_d[3][fi * BLK:(fi + 1) * BLK, :])
                w3_t.append(t)
            b_t = []
            for l in range(4):
                t = pc.tile([1, D if l < 3 else DOUT], f32, tag=f"b{l}")
                nc.sync.dma_start(out=t[:], in_=b_d[l][:])
                b_t.append(t)

            # iota [128, MB, 128] (value = inner index) and identity matrix
            iota_t = pc.tile([128, MB, BLK], i16, tag="iota")
            nc.gpsimd.iota(iota_t[:], pattern=[[0, MB], [1, BLK]], base=0,
                           channel_multiplier=0)
            pm_t = pc.tile([128, BLK], i16, tag="pmf")
            nc.gpsimd.iota(pm_t[:], pattern=[[-1, BLK]], base=0, channel_multiplier=1)
            id_bf = pc.tile([128, BLK], bf16, tag="idbf")
            nc.vector.tensor_scalar(id_bf[:], pm_t[:], 0, None, OP.is_equal)

            outsb = pc.tile([DOUT, NLOC], f32, tag="outsb")

            h_t = [None] * NB      # current-layer activations per block (bf16)
            agg_t = [None] * NB    # f32 aggregation accumulators

            def rows_of(b):
                return last_rows if b == NB - 1 else BLK

            def slice_of_block(b):
                for s in range(NS):
                    if slice_blocks[s] <= b < slice_blocks[s + 1]:
                        return s
                raise AssertionError

            # regions / ag inputs per layer generation (pool bufs=2 rotates)
            def new_comm_tiles():
                agin = [pd.tile([slice_lens[s], D], bf16, tag=f"agin{s}", name=f"agin{s}")
                        for s in range(NS)]
                region = [pd.tile([C * slice_lens[s], D], bf16, tag=f"region{s}",
                                  addr_space="Shared", name=f"region{s}") for s in range(NS)]
                return agin, region

            def emit_gemm_scale(l, b, agin):
                """S_l[b] = dis_b * (h @ W_l)  (bf16) -> DMA into agin slice.
                l == 3 skips the GEMM (S_3 = dis*h3)."""
                s = slice_of_block(b)
                r0 = (b - slice_blocks[s]) * BLK
                rows = rows_of(b)
                sblk = pw.tile([BLK, D], bf16, tag="sblk")
                if l == 3:
                    nc.scalar.activation(sblk[:], h_t[b][:], FT.Copy,
                                         scale=dis_t[:, b:b + 1])
                else:
                    # lhsT tiles [f_in_half, n]
                    if l == 0:
                        hT = []
                        for fi in range(NFI):
                            t = pw.tile([BLK, BLK], bf16, tag=f"hT{fi}")
                            nc.sync.dma_start(
                                out=t[:],
                                in_=xT[fi * BLK:(fi + 1) * BLK, b * BLK:(b + 1) * BLK])
                            hT.append(t)
                    else:
                        hT = []
                        for fi in range(NFI):
                            tp = ppt.tile([BLK, BLK], bf16, tag="tp")
                            nc.tensor.transpose(tp[:], h_t[b][:, fi * BLK:(fi + 1) * BLK],
                                                id_bf[:])
                            t = pw.tile([BLK, BLK], bf16, tag=f"hT{fi}")
                            nc.scalar.copy(t[:], tp[:])
                            hT.append(t)
                    gp = ppg.tile([BLK, D], f32, tag="gp")
                    for fi in range(NFI):
                        nc.tensor.matmul(gp[:], hT[fi][:], w_t[l][fi][:],
                                         start=(fi == 0), stop=(fi == NFI - 1))
                    nc.scalar.activation(sblk[:], gp[:], FT.Copy,
                                         scale=dis_t[:, b:b + 1])
                nc.sync.dma_start(out=agin[s][r0:r0 + rows, :], in_=sblk[:rows, :])

            def emit_ag(s, agin, region):
                nc.gpsimd.collective_compute(
                    "AllGather", OP.bypass, replica_groups=RG,
                    ins=[agin[s][:]], outs=[region[s][:]])

            # ---------------- layer 0 phase A
            agin_cur, region_cur = new_comm_tiles()
            for b in range(NB):
                emit_gemm_scale(0, b, agin_cur)
                if b + 1 in slice_blocks:
                    emit_ag(slice_of_block(b), agin_cur, region_cur)

            # ---------------- layers: edge pass (+ fused next-layer prep)
            for l in range(4):
                agin_next, region_next = (new_comm_tiles() if l < 3 else (None, None))
                # gather + M-build + matmul streams
                g_tiles = {}   # chunk_off -> (tile, chunk_off, n)
                for (s, coff, n) in gather_insts:
                    gt = pg.tile([128, n, D], bf16, tag="g")
                    nc.gpsimd.dma_gather(
                        out_ap=gt[:],
                        in_ap=region_cur[s][:],
                        idxs_ap=idx_t[:, coff * 8:(coff + n) * 8],
                        num_idxs=n * BLK,
                        num_idxs_reg=n * BLK,
                        elem_size=D,
                    )
                    g_tiles[coff] = (gt, coff, n)
                m_tiles = {}   # mb batch index -> tile
                n_mb = -(-tch // MB)
                for k in range(n_mb):
                    nchk = min(MB, tch - k * MB)
                    mt = pm.tile([128, nchk, BLK], bf16, tag="m")
                    nc.vector.tensor_tensor(
                        mt[:],
                        iota_t[:, :nchk, :],
                        dloc_t[:, k * MB:k * MB + nchk]
                            .broadcast_to([128, nchk, BLK]),
                        OP.is_equal)
                    m_tiles[k] = mt

                def g_slice(j):
                    for (gt, coff, n) in g_tiles.values():
                        if coff <= j < coff + n:
                            return gt[:, j - coff, :]
                    raise AssertionError

                def m_slice(j):
                    return m_tiles[j // MB][:, j % MB, :]

                for s in range(NS):
                    for b in range(NB):
                        c0 = int(group_off[s, b])
                        ncks = int(n_chunks[s, b])
                        ap_ = ppa.tile([BLK, D], f32, tag="aggp")
                        for j in range(c0, c0 + ncks):
                            is_last = (j == c0 + ncks - 1)
                            add_bias = is_last and s == NS - 1 and l < 3
                            nc.tensor.matmul(ap_[:], m_slice(j), g_slice(j),
                                             start=(j == c0),
                                             stop=(is_last and not add_bias))
                            if add_bias:
                                nc.tensor.matmul(
                                    ap_[:],
                                    invdis_t[0:1, b * BLK:(b + 1) * BLK],
                                    b_t[l][:], start=False, stop=True)
                        if s == 0:
                            agg_t[b] = pa.tile([BLK, D], f32, tag=f"agg{b}", name=f"agg{b}")
                            nc.vector.tensor_copy(agg_t[b][:], ap_[:])
                        else:
                            nc.vector.tensor_add(agg_t[b][:], agg_t[b][:], ap_[:])

                        if s == NS - 1:
                            # epilogue for block b
                            if l < 3:
                                h_t[b] = ph.tile([BLK, D], bf16, tag=f"h{b}", name=f"h{b}")
                                nc.scalar.activation(h_t[b][:], agg_t[b][:], FT.Relu,
                                                     scale=dis_t[:, b:b + 1])
                                emit_gemm_scale(l + 1, b, agin_next)
                                if b + 1 in slice_blocks:
                                    emit_ag(slice_of_block(b), agin_next, region_next)
                            else:
                                tblk = pw.tile([BLK, D], bf16, tag="sblk")
                                nc.scalar.activation(tblk[:], agg_t[b][:], FT.Copy,
                                                     scale=dis_t[:, b:b + 1])
                                tT = []
                                for fi in range(NFI):
                                    tp = ppt.tile([BLK, BLK], bf16, tag="tp")
                                    nc.tensor.transpose(
                                        tp[:], tblk[:, fi * BLK:(fi + 1) * BLK], id_bf[:])
                                    t = pw.tile([BLK, BLK], bf16, tag=f"hT{fi}")
                                    nc.scalar.copy(t[:], tp[:])
                                    tT.append(t)
                                op_ = ppa.tile([DOUT, BLK], f32, tag="aggp")
                                for fi in range(NFI):
                                    nc.tensor.matmul(op_[:], w3_t[fi][:], tT[fi][:],
                                                     start=(fi == 0), stop=False)
                                nc.tensor.matmul(op_[:], b_t[3][:], ones_t[:],
                                                 start=False, stop=True)
                                rows = rows_of(b)
                                nc.scalar.copy(
                                    outsb[:, b * BLK:b * BLK + rows], op_[:, :rows])
                if l < 3:
                    agin_cur, region_cur = agin_next, region_next

            nc.sync.dma_start(out=outT[:], in_=outsb[:])

    nc.compile()
    return nc


# ----------------------------------------------------------------- driver ----


def _gather_full_inputs(cfg, inputs):
    x = np.asarray(inputs["x"], np.float32)
    W = [np.asarray(inputs[f"W{l}"], np.float32) for l in range(4)]
    B = [np.asarray(inputs[f"b{l}"], np.float32) for l in range(4)]
    return x, W, B


def run(cfg, inputs, runner):
    """runner(nc, in_maps) -> list of {name: np.ndarray} per core."""
    meta, per_core = preprocess(cfg, np.asarray(inputs["edge_index"]))
    x, W, B = _gather_full_inputs(cfg, inputs)
    in_maps = make_in_maps(cfg, meta, per_core, x, W, B)
    nc = build(cfg, meta)
    results = runner(nc, in_maps)
    outs = [np.asarray(r["outT"], np.float32).T for r in results]  # [NLOC, DOUT]
    return np.concatenate(outs, axis=0)


def _hw_runner(nc, in_maps):
    import sys
    if "/opt/trn_rl_repo" not in sys.path:
        sys.path.insert(0, "/opt/trn_rl_repo")
    from concourse import bass_utils
    res = bass_utils.run_bass_kernel_spmd(nc, in_maps, core_ids=list(range(len(in_maps))))
    return res.results


def kernel(**inputs):
    return run(CFG, inputs, _hw_runner)


# revision 7
# speedup vs baseline: 1.8080x; 1.7451x over previous
"""4-layer GCN on 8 Trainium2 NeuronCores.

Strategy (destination/node sharding):
  - Nodes row-sharded across 8 cores (6250 rows each). Edges owned by their
    destination core. Weights replicated.
  - Per layer L in {0,1,2}:  P = h @ W_L (local GEMM, bf16),
    S = dis * P  (dis = deg^-1/2, row scale, bf16)  -> AllGather(S) ->
    agg_d = sum_{e: dst=d} S[src_e]   (gather + one-hot matmul accumulate)
    h' = relu(dis_d * agg_d + b_L)
    Layer 3 commutes the GEMM past the aggregation (OUT_DIM=2 gathers would be
    tiny/inefficient): S = dis*h3, agg, out = (dis_d*agg) @ W3 + b3.
  - The edge aggregation runs as: dma_gather of 128-row chunks of S (bf16,
    512B rows, full DMA rate), then PE matmul  psum += M_chunk^T @ msgs_chunk
    where M[p, f] = (dstloc[p] == f) is built on the DVE from an iota +
    int16 compare (edges pre-sorted by destination block on the host).
  - AllGather is split into 4 block-aligned row slices per layer so the edge
    pass pipelines against the collective, and so each gather region stays
    under the int16 index limit of dma_gather.
"""

import math
import numpy as np
import ml_dtypes

BF16 = ml_dtypes.bfloat16

# ---------------------------------------------------------------- config ----


def make_cfg(n, e, d, dout, n_cores, slice_blocks):
    blk = 128
    nloc = n // n_cores
    nb = math.ceil(nloc / blk)
    assert slice_blocks[0] == 0 and slice_blocks[-1] == nb
    row_starts = [min(b * blk, nloc) for b in slice_blocks]
    slice_lens = [row_starts[i + 1] - row_starts[i] for i in range(len(row_starts) - 1)]
    return dict(
        N=n, E=e, D=d, DOUT=dout, C=n_cores, BLK=blk, NLOC=nloc, NB=nb,
        NPAD=nb * blk,
        SLICE_BLOCKS=slice_blocks,          # block index bounds per slice
        ROW_STARTS=row_starts[:-1],         # local-row start per slice
        SLICE_LENS=slice_lens,              # local rows per slice
        NS=len(slice_lens),
        GATHER_CHUNKS=8,                   # chunks (of 128 edges) per dma_gather
        MB=16,                              # chunks per M-build batch
    )


CFG = make_cfg(50000, 800000, 256, 2, 8, [0, 25, 49])

# ---------------------------------------------------------- host preprocess --


def preprocess(cfg, edge_index):
    """Sort/pad edges per core; build gather-index + dstloc streams.

    Returns (shared_meta, per_core_arrays).
    """
    N, C, NLOC, BLK, NB, NS = (cfg[k] for k in ("N", "C", "NLOC", "BLK", "NB", "NS"))
    row_starts = np.array(cfg["ROW_STARTS"], np.int64)
    slice_lens = np.array(cfg["SLICE_LENS"], np.int64)

    src = np.concatenate([np.asarray(edge_index[0], np.int64), np.arange(N)])
    dst = np.concatenate([np.asarray(edge_index[1], np.int64), np.arange(N)])
    deg = np.bincount(dst, minlength=N).astype(np.float32)
    dis = deg ** -0.5

    core = dst // NLOC
    per_core_raw = []
    counts = np.zeros((C, NS, NB), np.int64)
    for c in range(C):
        m = core == c
        s_e, d_e = src[m], dst[m] - c * NLOC
        b_e = d_e // BLK
        loc_e = d_e % BLK
        o_e = s_e // NLOC
        r_e = s_e % NLOC
        sl_e = np.searchsorted(row_starts, r_e, side="right") - 1
        gidx = o_e * slice_lens[sl_e] + (r_e - row_starts[sl_e])
        key = sl_e * NB + b_e
        order = np.argsort(key, kind="stable")
        per_core_raw.append((key[order], gidx[order], loc_e[order]))
        cnt = np.bincount(key, minlength=NS * NB)
        counts[c] = cnt.reshape(NS, NB)

    # uniform chunk counts across cores (SPMD: one program)
    n_chunks = np.maximum(1, -(-counts.max(axis=0) // BLK))  # [NS, NB]
    chunks_per_slice = n_chunks.sum(axis=1)                  # [NS]
    tch = int(chunks_per_slice.sum())
    total_idx = tch * BLK

    # chunk-group offsets (in chunks) per (s, b)
    group_off = np.zeros((NS, NB), np.int64)
    acc = 0
    for s in range(NS):
        for b in range(NB):
            group_off[s, b] = acc
            acc += n_chunks[s, b]

    # gather instruction split: per slice, pieces of <= GATHER_CHUNKS chunks
    gather_insts = []  # (slice, chunk_off, n_chunk)
    for s in range(NS):
        start = int(group_off[s, 0])
        rem = int(chunks_per_slice[s])
        off = start
        while rem > 0:
            n = min(rem, cfg["GATHER_CHUNKS"])
            gather_insts.append((s, off, n))
            off += n
            rem -= n

    per_core = []
    for c in range(C):
        key_s, gidx_s, loc_s = per_core_raw[c]
        idx_stream = np.zeros(total_idx, np.int16)
        dloc_stream = np.full(total_idx, -1, np.int16)
        # place each (s,b) group at its padded offset
        grp_sizes = np.bincount(key_s, minlength=NS * NB)
        pos = 0
        for g in range(NS * NB):
            sz = int(grp_sizes[g])
            s_, b_ = g // NB, g % NB
            o0 = int(group_off[s_, b_]) * BLK
            idx_stream[o0:o0 + sz] = gidx_s[pos:pos + sz]
            dloc_stream[o0:o0 + sz] = loc_s[pos:pos + sz]
            pos += sz
        # wrap: position i -> [i % 16, i // 16], replicate to 128 partitions
        idx_w = np.tile(idx_stream.reshape(-1, 16).T, (8, 1))  # [128, total/16]
        # dstloc: chunk j, partition p  -> edge j*128+p
        dloc_w = dloc_stream.reshape(-1, BLK).T.copy()         # [128, tch]
        per_core.append(dict(idx=idx_w, dstloc=dloc_w))

    meta = dict(n_chunks=n_chunks, group_off=group_off, gather_insts=gather_insts,
                tch=tch, total_idx=total_idx, dis=dis)
    return meta, per_core


def make_in_maps(cfg, meta, per_core, x, W, B):
    C, NLOC, NB, BLK, NPAD, D, DOUT = (cfg[k] for k in
                                       ("C", "NLOC", "NB", "BLK", "NPAD", "D", "DOUT"))
    dis = meta["dis"]
    in_maps = []
    for c in range(C):
        xc = np.asarray(x[c * NLOC:(c + 1) * NLOC], np.float32)
        xT = np.zeros((D, NPAD), BF16)
        xT[:, :NLOC] = xc.T.astype(BF16)
        dis_c = dis[c * NLOC:(c + 1) * NLOC]
        dis_w = np.ones((BLK, NB), np.float32)
        dw = np.ones(NPAD, np.float32)
        dw[:NLOC] = dis_c
        dis_w[:, :] = dw.reshape(NB, BLK).T
        invdis = np.zeros((1, NPAD), np.float32)
        invdis[0, :NLOC] = 1.0 / dis_c
        m = dict(
            xT=xT,
            idx=per_core[c]["idx"],
            dstloc=per_core[c]["dstloc"],
            disw=dis_w,
            invdis=invdis,
            ones=np.ones((1, BLK), np.float32),
            w0=np.asarray(W[0], np.float32).astype(BF16),
            w1=np.asarray(W[1], np.float32).astype(BF16),
            w2=np.asarray(W[2], np.float32).astype(BF16),
            w3=np.asarray(W[3], np.float32).astype(BF16),
            b0=np.asarray(B[0], np.float32).reshape(1, D),
            b1=np.asarray(B[1], np.float32).reshape(1, D),
            b2=np.asarray(B[2], np.float32).reshape(1, D),
            b3=np.asarray(B[3], np.float32).reshape(1, DOUT),
        )
        in_maps.append(m)
    return in_maps


# ---------------------------------------------------------------- builder ----


def build(cfg, meta):
    import sys
    if "/opt/trn_rl_repo" not in sys.path:
        sys.path.insert(0, "/opt/trn_rl_repo")
    import concourse.bass as bass
    from concourse import bacc, tile, mybir

    C, D, DOUT, BLK, NB, NS, NLOC, NPAD = (cfg[k] for k in
        ("C", "D", "DOUT", "BLK", "NB", "NS", "NLOC", "NPAD"))
    MB, GCH = cfg["MB"], cfg["GATHER_CHUNKS"]
    n_chunks, group_off = meta["n_chunks"], meta["group_off"]
    gather_insts, tch = meta["gather_insts"], meta["tch"]
    slice_lens, slice_blocks = cfg["SLICE_LENS"], cfg["SLICE_BLOCKS"]
    NFI = D // BLK  # f_in halves (2)
    f32, bf16, i16 = mybir.dt.float32, mybir.dt.bfloat16, mybir.dt.int16
    FT = mybir.ActivationFunctionType
    OP = mybir.AluOpType
    RG = [list(range(C))]

    nc = bacc.Bacc("TRN2", target_bir_lowering=False, debug=False, num_devices=C)

    xT = nc.dram_tensor("xT", [D, NPAD], bf16, kind="ExternalInput")
    idx_d = nc.dram_tensor("idx", [128, meta["total_idx"] // 16], i16, kind="ExternalInput")
    dloc_d = nc.dram_tensor("dstloc", [128, tch], i16, kind="ExternalInput")
    disw_d = nc.dram_tensor("disw", [BLK, NB], f32, kind="ExternalInput")
    invdis_d = nc.dram_tensor("invdis", [1, NPAD], f32, kind="ExternalInput")
    ones_d = nc.dram_tensor("ones", [1, BLK], f32, kind="ExternalInput")
    w_d = [nc.dram_tensor(f"w{l}", [D, D if l < 3 else DOUT], bf16, kind="ExternalInput")
           for l in range(4)]
    b_d = [nc.dram_tensor(f"b{l}", [1, D if l < 3 else DOUT], f32, kind="ExternalInput")
           for l in range(4)]
    outT = nc.dram_tensor("outT", [DOUT, NLOC], f32, kind="ExternalOutput")

    last_rows = NLOC - (NB - 1) * BLK  # rows in final block

    with tile.TileContext(nc) as tc:
        with tc.tile_pool(name="const", bufs=1) as pc, \
             tc.tile_pool(name="hpool", bufs=1) as ph, \
             tc.tile_pool(name="aggpool", bufs=1) as pa, \
             tc.tile_pool(name="work", bufs=3) as pw, \
             tc.tile_pool(name="gpool", bufs=3) as pg, \
             tc.tile_pool(name="mpool", bufs=3) as pm, \
             tc.tile_pool(name="psA", bufs=3, space="PSUM") as ppa, \
             tc.tile_pool(name="psT", bufs=2, space="PSUM") as ppt, \
             tc.tile_pool(name="psG", bufs=2, space="PSUM") as ppg, \
             tc.tile_pool(name="dram", bufs=2, space="DRAM") as pd:

            # ---- constants into SBUF
            idx_t = pc.tile([128, meta["total_idx"] // 16], i16, tag="idx")
            nc.sync.dma_start(out=idx_t[:], in_=idx_d[:])
            dloc_t = pc.tile([128, tch], i16, tag="dloc")
            nc.sync.dma_start(out=dloc_t[:], in_=dloc_d[:])
            dis_t = pc.tile([BLK, NB], f32, tag="disw")
            nc.sync.dma_start(out=dis_t[:], in_=disw_d[:])
            invdis_t = pc.tile([1, NPAD], f32, tag="invdis")
            nc.sync.dma_start(out=invdis_t[:], in_=invdis_d[:])
            ones_t = pc.tile([1, BLK], f32, tag="ones")
            nc.sync.dma_start(out=ones_t[:], in_=ones_d[:])
            w_t = []
            for l in range(3):
                tiles = []
                for fi in range(NFI):
                    t = pc.tile([BLK, D], bf16, tag=f"w{l}_{fi}")
                    nc.sync.dma_start(out=t[:], in_=w_d[l][fi * BLK:(fi + 1) * BLK, :])
                    tiles.append(t)
                w_t.append(tiles)
            w3_t = []
            for fi in range(NFI):
                t = pc.tile([BLK, DOUT], bf16, tag=f"w3_{fi}")
                nc.sync.dma_start(out=t[:], in_=w_d[3][fi * BLK:(fi + 1) * BLK, :])
                w3_t.append(t)
            b_t = []
            for l in range(4):
                t = pc.tile([1, D if l < 3 else DOUT], f32, tag=f"b{l}")
                nc.sync.dma_start(out=t[:], in_=b_d[l][:])
                b_t.append(t)

            # iota [128, MB, 128] (value = inner index) and identity matrix
            iota_t = pc.tile([128, MB, BLK], i16, tag="iota")
            nc.gpsimd.iota(iota_t[:], pattern=[[0, MB], [1, BLK]], base=0,
                           channel_multiplier=0)
            pm_t = pc.tile([128, BLK], i16, tag="pmf")
            nc.gpsimd.iota(pm_t[:], pattern=[[-1, BLK]], base=0, channel_multiplier=1)
            id_bf = pc.tile([128, BLK], bf16, tag="idbf")
            nc.vector.tensor_scalar(id_bf[:], pm_t[:], 0, None, OP.is_equal)

            outsb = pc.tile([DOUT, NLOC], f32, tag="outsb")

            h_t = [None] * NB      # current-layer activations per block (bf16)
            agg_t = [None] * NB    # f32 aggregation accumulators

            def rows_of(b):
                return last_rows if b == NB - 1 else BLK

            def slice_of_block(b):
                for s in range(NS):
                    if slice_blocks[s] <= b < slice_blocks[s + 1]:
                        return s
                raise AssertionError

            # regions / ag inputs per layer generation (pool bufs=2 rotates)
            def new_comm_tiles():
                agin = [pd.tile([slice_lens[s], D], bf16, tag=f"agin{s}", name=f"agin{s}")
                        for s in range(NS)]
                region = [pd.tile([C * slice_lens[s], D], bf16, tag=f"region{s}",
                                  addr_space="Shared", name=f"region{s}") for s in range(NS)]
                return agin, region

            def emit_gemm_scale(l, b, agin):
                """S_l[b] = dis_b * (h @ W_l)  (bf16) -> DMA into agin slice.
                l == 3 skips the GEMM (S_3 = dis*h3)."""
                s = slice_of_block(b)
                r0 = (b - slice_blocks[s]) * BLK
                rows = rows_of(b)
                sblk = pw.tile([BLK, D], bf16, tag="sblk")
                if l == 3:
                    nc.scalar.activation(sblk[:], h_t[b][:], FT.Copy,
                                         scale=dis_t[:, b:b + 1])
                else:
                    # lhsT tiles [f_in_half, n]
                    if l == 0:
                        hT = []
                        for fi in range(NFI):
                            t = pw.tile([BLK, BLK], bf16, tag=f"hT{fi}")
                            nc.sync.dma_start(
                                out=t[:],
                                in_=xT[fi * BLK:(fi + 1) * BLK, b * BLK:(b + 1) * BLK])
                            hT.append(t)
                    else:
                        hT = []
                        for fi in range(NFI):
                            tp = ppt.tile([BLK, BLK], bf16, tag="tp")
                            nc.tensor.transpose(tp[:], h_t[b][:, fi * BLK:(fi + 1) * BLK],
                                                id_bf[:])
                            t = pw.tile([BLK, BLK], bf16, tag=f"hT{fi}")
                            nc.scalar.copy(t[:], tp[:])
                            hT.append(t)
                    gp = ppg.tile([BLK, D], f32, tag="gp")
                    for fi in range(NFI):
                        nc.tensor.matmul(gp[:], hT[fi][:], w_t[l][fi][:],
                                         start=(fi == 0), stop=(fi == NFI - 1))
                    nc.scalar.activation(sblk[:], gp[:], FT.Copy,
                                         scale=dis_t[:, b:b + 1])
                nc.sync.dma_start(out=agin[s][r0:r0 + rows, :], in_=sblk[:rows, :])

            def emit_ag(s, agin, region):
                nc.gpsimd.collective_compute(
                    "AllGather", OP.bypass, replica_groups=RG,
                    ins=[agin[s][:]], outs=[region[s][:]])

            # ---------------- layer 0 phase A
            agin_cur, region_cur = new_comm_tiles()
            for b in range(NB):
                emit_gemm_scale(0, b, agin_cur)
                if b + 1 in slice_blocks:
                    emit_ag(slice_of_block(b), agin_cur, region_cur)

            # ---------------- layers: edge pass (+ fused next-layer prep)
            for l in range(4):
                agin_next, region_next = (new_comm_tiles() if l < 3 else (None, None))
                # gather + M-build + matmul streams
                g_tiles = {}   # chunk_off -> (tile, chunk_off, n)
                for (s, coff, n) in gather_insts:
                    gt = pg.tile([128, n, D], bf16, tag="g")
                    nc.gpsimd.dma_gather(
                        out_ap=gt[:],
                        in_ap=region_cur[s][:],
                        idxs_ap=idx_t[:, coff * 8:(coff + n) * 8],
                        num_idxs=n * BLK,
                        num_idxs_reg=n * BLK,
                        elem_size=D,
                    )
                    g_tiles[coff] = (gt, coff, n)
                m_tiles = {}   # mb batch index -> tile
                n_mb = -(-tch // MB)
                for k in range(n_mb):
                    nchk = min(MB, tch - k * MB)
                    mt = pm.tile([128, nchk, BLK], bf16, tag="m")
                    nc.vector.tensor_tensor(
                        mt[:],
                        iota_t[:, :nchk, :],
                        dloc_t[:, k * MB:k * MB + nchk]
                            .broadcast_to([128, nchk, BLK]),
                        OP.is_equal)
                    m_tiles[k] = mt

                def g_slice(j):
                    for (gt, coff, n) in g_tiles.values():
                        if coff <= j < coff + n:
                            return gt[:, j - coff, :]
                    raise AssertionError

                def m_slice(j):
                    return m_tiles[j // MB][:, j % MB, :]

                for s in range(NS):
                    for b in range(NB):
                        c0 = int(group_off[s, b])
                        ncks = int(n_chunks[s, b])
                        ap_ = ppa.tile([BLK, D], f32, tag="aggp")
                        for j in range(c0, c0 + ncks):
                            is_last = (j == c0 + ncks - 1)
                            add_bias = is_last and s == NS - 1 and l < 3
                            nc.tensor.matmul(ap_[:], m_slice(j), g_slice(j),
                                             start=(j == c0),
                                             stop=(is_last and not add_bias))
                            if add_bias:
                                nc.tensor.matmul(
                                    ap_[:],
                                    invdis_t[0:1, b * BLK:(b + 1) * BLK],
                                    b_t[l][:], start=False, stop=True)
                        if s == 0:
                            agg_t[b] = pa.tile([BLK, D], f32, tag=f"agg{b}", name=f"agg{b}")
                            nc.vector.tensor_copy(agg_t[b][:], ap_[:])
                        else:
                            nc.vector.tensor_add(agg_t[b][:], agg_t[b][:], ap_[:])

                        if s == NS - 1:
                            # epilogue for block b
                            if l < 3:
                                h_t[b] = ph.tile([BLK, D], bf16, tag=f"h{b}", name=f"h{b}")
                                nc.scalar.activation(h_t[b][:], agg_t[b][:], FT.Relu,
                                                     scale=dis_t[:, b:b + 1])
                                emit_gemm_scale(l + 1, b, agin_next)
                                if b + 1 in slice_blocks:
                                    emit_ag(slice_of_block(b), agin_next, region_next)
                            else:
                                tblk = pw.tile([BLK, D], bf16, tag="sblk")
                                nc.scalar.activation(tblk[:], agg_t[b][:], FT.Copy,
                                                     scale=dis_t[:, b:b + 1])
                                tT = []
                                for fi in range(NFI):
                                    tp = ppt.tile([BLK, BLK], bf16, tag="tp")
                                    nc.tensor.transpose(
                                        tp[:], tblk[:, fi * BLK:(fi + 1) * BLK], id_bf[:])
                                    t = pw.tile([BLK, BLK], bf16, tag=f"hT{fi}")
                                    nc.scalar.copy(t[:], tp[:])
                                    tT.append(t)
                                op_ = ppa.tile([DOUT, BLK], f32, tag="aggp")
                                for fi in range(NFI):
                                    nc.tensor.matmul(op_[:], w3_t[fi][:], tT[fi][:],
                                                     start=(fi == 0), stop=False)
                                nc.tensor.matmul(op_[:], b_t[3][:], ones_t[:],
                                                 start=False, stop=True)
                                rows = rows_of(b)
                                nc.scalar.copy(
                                    outsb[:, b * BLK:b * BLK + rows], op_[:, :rows])
                if l < 3:
                    agin_cur, region_cur = agin_next, region_next

            nc.sync.dma_start(out=outT[:], in_=outsb[:])

    nc.compile()
    return nc


# ----------------------------------------------------------------- driver ----


def _gather_full_inputs(cfg, inputs):
    x = np.asarray(inputs["x"], np.float32)
    W = [np.asarray(inputs[f"W{l}"], np.float32) for l in range(4)]
    B = [np.asarray(inputs[f"b{l}"], np.float32) for l in range(4)]
    return x, W, B


def run(cfg, inputs, runner):
    """runner(nc, in_maps) -> list of {name: np.ndarray} per core."""
    meta, per_core = preprocess(cfg, np.asarray(inputs["edge_index"]))
    x, W, B = _gather_full_inputs(cfg, inputs)
    in_maps = make_in_maps(cfg, meta, per_core, x, W, B)
    nc = build(cfg, meta)
    results = runner(nc, in_maps)
    outs = [np.asarray(r["outT"], np.float32).T for r in results]  # [NLOC, DOUT]
    return np.concatenate(outs, axis=0)


def _hw_runner(nc, in_maps):
    import sys
    if "/opt/trn_rl_repo" not in sys.path:
        sys.path.insert(0, "/opt/trn_rl_repo")
    from concourse import bass_utils
    res = bass_utils.run_bass_kernel_spmd(nc, in_maps, core_ids=list(range(len(in_maps))))
    return res.results


def kernel(**inputs):
    return run(CFG, inputs, _hw_runner)
